# revision 24
# baseline (speedup 1.0000x reference)
"""MoE routing kernel for Trainium2 (8 NeuronCores, SPMD expert-parallel).

Contract: kernel(**full_inputs) -> full output [B, S, H] float32.

Strategy
--------
- Host: compute the (tiny) gate + group-topk routing in numpy (bit-identical
  selection to the jax reference), build the per-(token,expert) combine
  weights, and dispatch: gather each expert's tokens into a padded,
  transposed bf16 buffer.  This is the "all-to-all by topk_idx" of the
  sharding hint, done at input-sharding time.
- Device (SPMD over 8 cores): core c holds experts (2c, 2c+1) and a 1/8
  column-slice of the shared expert.  Each core runs the SwiGLU MLP for its
  two experts over their gathered tokens (unweighted), plus its shared
  slice over all tokens, producing partial outputs in [H, tokens] layout.
- Host: scale per-expert outputs by routing weights, scatter-add over
  token indices, add the 8 shared partials, transpose back.

All matmuls run in bf16 with fp32 PSUM accumulation.  Weights stream from
HBM exactly once per kernel (strided 3D DMAs put the [H, .] panels into
[128, H/128, .] SBUF tiles); token tiles stay k-resident; gate/up chains
are emitted sequentially per token-slice so PSUM slots recycle without
stalling the PE.
"""

import math

import numpy as np
import ml_dtypes

H = 2048          # hidden size
I = 1408          # intermediate per routed expert
E = 16            # routed experts
G = 4             # groups
TOPK_GROUP = 2
TOP_K = 6
N_SHARED = 2
SCALE_FACTOR = 2.5
SI = I * N_SHARED  # 2816 shared intermediate
N_CORES = 8
EXP_PER_CORE = E // N_CORES  # 2
S_SLICE_RAW = SI // N_CORES  # 352
S_SLICE = 384                # padded to 3*128
P = 128
BF16 = ml_dtypes.bfloat16

_COMPILED = {}  # (T, C_cap, w) -> nc
_LAST = {}      # debug/profiling handle for test.py


def _gate_host(hs, gate_weight, bias):
    """numpy replica of reference._gate (verified bit-identical selection)."""
    T = hs.shape[0]
    logits = hs @ gate_weight.T                       # [T, E] fp32
    scores = 1.0 / (1.0 + np.exp(-logits))
    sfc = scores + bias[None, :]
    gs = sfc.reshape(T, G, E // G)
    gsort = np.sort(gs, axis=-1)
    group_scores = gsort[..., -1] + gsort[..., -2]
    group_idx = np.argsort(-group_scores, axis=-1, kind="stable")[:, :TOPK_GROUP]
    gmask = np.zeros((T, G), bool)
    gmask[np.arange(T)[:, None], group_idx] = True
    smask = np.repeat(gmask, E // G, axis=1)
    tmp = np.where(smask, sfc, 0.0)
    topk_idx = np.argsort(-tmp, axis=-1, kind="stable")[:, :TOP_K]
    topk_w = np.take_along_axis(scores, topk_idx, axis=1)
    topk_w = topk_w / (topk_w.sum(-1, keepdims=True) + 1e-20) * SCALE_FACTOR
    return topk_idx.astype(np.int32), topk_w.astype(np.float32)


def _build(T, caps):
    """Build + compile the SPMD Bass program.

    T    : total tokens (every core sees all of them for its shared slice)
    caps : per expert slot, (C_cap, w): gathered-token capacity and matmul
           free-dim slice width; C_cap = NP_R * 2 * w
    """
    import concourse.mybir as mybir
    import concourse.tile as tile
    from concourse import bacc

    bf = mybir.dt.bfloat16
    f32 = mybir.dt.float32
    AF = mybir.ActivationFunctionType

    KH = H // P        # 16 contraction chunks over H
    MI = I // P        # 11 I chunks
    MH = H // P        # 16 output H chunks
    MS = S_SLICE // P  # 3
    NP_S = T // 1024   # shared token blocks (2 x 512 slices each)
    for (C_cap, w) in caps:
        assert C_cap % (2 * w) == 0 and w <= 512
    C_tot = sum(C_cap for C_cap, _ in caps)
    slot_base = [sum(C for C, _ in caps[:s]) for s in range(len(caps))]

    nc = bacc.Bacc("TRN2", target_bir_lowering=False, debug=False,
                   num_devices=N_CORES)
    xs = nc.dram_tensor("xs", [H, T], bf, kind="ExternalInput")
    xg = nc.dram_tensor("xg", [H, C_tot], bf, kind="ExternalInput")
    # weight panels are pre-tiled on the host to the exact SBUF tile layout
    # [tile_idx, partition, ko*128+c] so every load is a contiguous
    # per-partition stream
    wg = nc.dram_tensor("wg", [EXP_PER_CORE * MI, P, KH * P], bf,
                        kind="ExternalInput")
    wu = nc.dram_tensor("wu", [EXP_PER_CORE * MI, P, KH * P], bf,
                        kind="ExternalInput")
    wd = nc.dram_tensor("wd", [EXP_PER_CORE * MH, P, MI * P], bf,
                        kind="ExternalInput")
    sg = nc.dram_tensor("sg", [MS, P, KH * P], bf, kind="ExternalInput")
    su = nc.dram_tensor("su", [MS, P, KH * P], bf, kind="ExternalInput")
    sd = nc.dram_tensor("sd", [P, MS * H], bf, kind="ExternalInput")
    ye = nc.dram_tensor("ye", [H, C_tot], bf, kind="ExternalOutput")
    ys = nc.dram_tensor("ys", [H, T], bf, kind="ExternalOutput")



    MGS = [(0, 4), (4, 4), (8, 3)]          # I chunk groups (11)
    MGS_D = [(0, 4), (4, 4), (8, 4), (12, 4)]  # H chunk groups (16)

    with tile.TileContext(nc) as tc:
        with (
            tc.tile_pool(name="xp", bufs=34) as xp,    # x tiles <=[128,1024] bf16
            tc.tile_pool(name="wp", bufs=6) as wp,     # [128,16,128] weight cols
            tc.tile_pool(name="wdp", bufs=4) as wdp,   # [128,11,128] down cols
            tc.tile_pool(name="sdp", bufs=1) as sdp,   # [128,3,2048] shared down
            tc.tile_pool(name="itp", bufs=46) as itp,  # [128,512] bf16 inter
            tc.tile_pool(name="tmp", bufs=4) as tmp,   # silu temp
            tc.tile_pool(name="otp", bufs=6) as otp,   # [128,1024] bf16 out
            tc.tile_pool(name="pg", bufs=2, space="PSUM") as pgp,
            tc.tile_pool(name="pu", bufs=2, space="PSUM") as pup,
            tc.tile_pool(name="py", bufs=4, space="PSUM") as pyp,
        ):
            # ---------------- shared expert (column slice) ----------------
            sdt = sdp.tile([P, MS, H], bf, name="sdt", tag="sdt")
            nc.scalar.dma_start(sdt[:], sd.ap().rearrange("p (ko c) -> p ko c", c=H))

            # spread the critical first block's loads over four queues so the
            # first matmul chain isn't gated on one sequencer issuing 16 DMAs
            first_engines = [nc.scalar, nc.sync, nc.gpsimd]
            for np_ in range(NP_S):
                c0 = np_ * 1024
                xst = []
                for k in range(KH):
                    t = xp.tile([P, 1024], bf, name=f"xs{np_}_{k}", tag="x")
                    eng = first_engines[k % 3] if np_ == 0 else nc.scalar
                    eng.dma_start(t[:], xs[k * P:(k + 1) * P, c0:c0 + 1024])
                    xst.append(t)
                sint = {}
                for m in range(MS):
                    mo = m * P
                    sgt = wp.tile([P, KH, P], bf, name=f"sgt{np_}_{m}", tag="wp")
                    nc.sync.dma_start(sgt[:], sg[m].rearrange("p (ko c) -> p ko c", c=P))
                    sut = wp.tile([P, KH, P], bf, name=f"sut{np_}_{m}", tag="wp")
                    nc.sync.dma_start(sut[:], su[m].rearrange("p (ko c) -> p ko c", c=P))
                    for j in range(2):
                        psg = pgp.tile([P, 512], f32, name=f"psgs{np_}_{m}{j}",
                                       tag="pg")
                        for k in range(KH):
                            nc.tensor.matmul(psg[:], sgt[:, k, :],
                                             xst[k][:, j * 512:(j + 1) * 512],
                                             start=(k == 0), stop=(k == KH - 1))
                        st = tmp.tile([P, 512], bf, name=f"sts{np_}_{m}{j}",
                                      tag="tmp")
                        nc.scalar.activation(st[:], psg[:], AF.Silu)
                        psu = pup.tile([P, 512], f32, name=f"psus{np_}_{m}{j}",
                                       tag="pu")
                        for k in range(KH):
                            nc.tensor.matmul(psu[:], sut[:, k, :],
                                             xst[k][:, j * 512:(j + 1) * 512],
                                             start=(k == 0), stop=(k == KH - 1))
                        it = itp.tile([P, 512], bf, name=f"si{np_}_{m}{j}",
                                      tag="it")
                        nc.vector.tensor_mul(it[:], st[:], psu[:])
                        sint[(m, j)] = it
                for M in range(MH):
                    ot = otp.tile([P, 1024], bf, name=f"ots{np_}_{M}", tag="ot")
                    for j in range(2):
                        psy = pyp.tile([P, 512], f32, name=f"psys{np_}_{M}{j}",
                                       tag="py")
                        for K in range(MS):
                            nc.tensor.matmul(psy[:], sdt[:, K, M * P:(M + 1) * P],
                                             sint[(K, j)][:],
                                             start=(K == 0), stop=(K == MS - 1))
                        nc.vector.tensor_copy(ot[:, j * 512:(j + 1) * 512], psy[:])
                    nc.gpsimd.dma_start(ys[M * P:(M + 1) * P, c0:c0 + 1024], ot[:])

            # ---------------- routed experts ----------------
            for s, (C_cap, w) in enumerate(caps):
                NP_R = C_cap // (2 * w)
                xgt = {}
                for np_ in range(NP_R):
                    b0 = slot_base[s] + np_ * 2 * w
                    for k in range(KH):
                        t = xp.tile([P, 2 * w], bf, name=f"xg{s}_{np_}_{k}",
                                    tag="x")
                        nc.scalar.dma_start(
                            t[:], xg[k * P:(k + 1) * P, b0:b0 + 2 * w])
                        xgt[(np_, k)] = t
                inter = {}
                for m in range(MI):
                    mo = s * I + m * P
                    wgt = wp.tile([P, KH, P], bf, name=f"wgt{s}_{m}", tag="wp")
                    nc.sync.dma_start(wgt[:], wg[s * MI + m].rearrange("p (ko c) -> p ko c", c=P))
                    wut = wp.tile([P, KH, P], bf, name=f"wut{s}_{m}", tag="wp")
                    nc.sync.dma_start(wut[:], wu[s * MI + m].rearrange("p (ko c) -> p ko c", c=P))
                    for np_ in range(NP_R):
                        for j in range(2):
                            psg = pgp.tile([P, 512], f32,
                                           name=f"psg{s}_{m}_{np_}{j}",
                                           tag="pg")
                            for k in range(KH):
                                nc.tensor.matmul(
                                    psg[:, :w], wgt[:, k, :],
                                    xgt[(np_, k)][:, j * w:(j + 1) * w],
                                    start=(k == 0), stop=(k == KH - 1))
                            st = tmp.tile([P, 512], bf,
                                          name=f"st{s}_{m}_{np_}{j}",
                                          tag="tmp")
                            nc.scalar.activation(st[:, :w], psg[:, :w],
                                                 AF.Silu)
                            psu = pup.tile([P, 512], f32,
                                           name=f"psu{s}_{m}_{np_}{j}",
                                           tag="pu")
                            for k in range(KH):
                                nc.tensor.matmul(
                                    psu[:, :w], wut[:, k, :],
                                    xgt[(np_, k)][:, j * w:(j + 1) * w],
                                    start=(k == 0), stop=(k == KH - 1))
                            it = itp.tile([P, 512], bf,
                                          name=f"it{s}_{m}_{np_}{j}",
                                          tag="it")
                            nc.vector.tensor_mul(it[:, :w], st[:, :w],
                                                 psu[:, :w])
                            inter[(m, np_, j)] = it
                for M in range(MH):
                    Mo = s * H + M * P
                    wdt = wdp.tile([P, MI, P], bf, name=f"wdt{s}_{M}", tag="wdt")
                    nc.sync.dma_start(wdt[:], wd[s * MH + M].rearrange("p (ko c) -> p ko c", c=P))
                    for np_ in range(NP_R):
                        b0 = slot_base[s] + np_ * 2 * w
                        ot = otp.tile([P, 1024], bf,
                                      name=f"ot{s}_{M}_{np_}", tag="ot")
                        for j in range(2):
                            psy = pyp.tile([P, 512], f32,
                                           name=f"psy{s}_{M}_{np_}{j}",
                                           tag="py")
                            for K in range(MI):
                                nc.tensor.matmul(
                                    psy[:, :w], wdt[:, K, :],
                                    inter[(K, np_, j)][:, :w],
                                    start=(K == 0), stop=(K == MI - 1))
                            nc.vector.tensor_copy(
                                ot[:, j * w:(j + 1) * w], psy[:, :w])
                        nc.gpsimd.dma_start(
                            ye[M * P:(M + 1) * P, b0:b0 + 2 * w],
                            ot[:, :2 * w])

    nc.compile()
    return nc


def _get_compiled(T, caps):
    key = (T, tuple(caps))
    if key not in _COMPILED:
        _COMPILED[key] = _build(T, caps)
    return _COMPILED[key]


def _cap_for(maxc):
    maxc = max(int(maxc), 64)
    np_r = max(2, math.ceil(maxc / 2048))
    w = min(512, math.ceil(maxc / (np_r * 2 * 4)) * 4)
    C_cap = np_r * 2 * w
    assert C_cap >= maxc
    return C_cap, w


def kernel(hidden_states, gate_weight, e_score_correction_bias,
           gate_proj, up_proj, down_proj,
           shared_gate_w, shared_up_w, shared_down_w):
    from concourse.bass_utils import run_bass_kernel_spmd

    hs = np.asarray(hidden_states, dtype=np.float32)
    B, S, Hh = hs.shape
    assert Hh == H
    hsf = np.ascontiguousarray(hs.reshape(-1, H))
    T = hsf.shape[0]
    gate_weight = np.asarray(gate_weight, np.float32)
    bias = np.asarray(e_score_correction_bias, np.float32)
    gate_proj = np.asarray(gate_proj, np.float32)
    up_proj = np.asarray(up_proj, np.float32)
    down_proj = np.asarray(down_proj, np.float32)
    shared_gate_w = np.asarray(shared_gate_w, np.float32)
    shared_up_w = np.asarray(shared_up_w, np.float32)
    shared_down_w = np.asarray(shared_down_w, np.float32)

    # ---- routing on host ----
    topk_idx, topk_w = _gate_host(hsf, gate_weight, bias)
    comb = np.zeros((T, E), np.float32)
    np.add.at(comb, (np.arange(T)[:, None], topk_idx), topk_w)
    sel = np.zeros((T, E), bool)
    sel[np.arange(T)[:, None], topk_idx] = True
    idx_e = [np.nonzero(sel[:, e])[0] for e in range(E)]
    counts = np.array([len(ix) for ix in idx_e])

    # assign experts to (core, slot): slot 0 gets the 8 largest, slot 1 the
    # 8 smallest, so each slot's capacity (uniform across cores under SPMD)
    # hugs its own max count
    order = np.argsort(-counts, kind="stable")
    assign = np.zeros((N_CORES, EXP_PER_CORE), np.int64)
    for c in range(N_CORES):
        assign[c, 0] = order[c]
        assign[c, 1] = order[2 * N_CORES - 1 - c]
    caps = [
        _cap_for(counts[assign[:, 0]].max()),
        _cap_for(counts[assign[:, 1]].max()),
    ]
    slot_base = [0, caps[0][0]]
    C_tot = caps[0][0] + caps[1][0]

    # ---- host-side dispatch (shard + transpose + bf16 cast) ----
    xsT = np.ascontiguousarray(hsf.T).astype(BF16)          # [H, T]

    MI, MH, MS, KH = I // P, H // P, S_SLICE // P, H // P

    def tile_gu(wmat):  # [I, H] -> [MI, P, KH*P] : (m, p_h, ko_h*P + c_i)
        return np.ascontiguousarray(
            wmat.reshape(MI, P, KH, P).transpose(0, 3, 2, 1)
        ).reshape(MI, P, KH * P).astype(BF16)

    def tile_dn(wmat):  # [H, I] -> [MH, P, MI*P] : (M, p_i, Ko_i*P + c_h)
        return np.ascontiguousarray(
            wmat.reshape(MH, P, MI, P).transpose(0, 3, 2, 1)
        ).reshape(MH, P, MI * P).astype(BF16)

    in_maps = []
    for c in range(N_CORES):
        e0, e1 = assign[c]
        xg_c = np.zeros((H, C_tot), BF16)
        for sslot, e in enumerate((e0, e1)):
            b0 = slot_base[sslot]
            xg_c[:, b0:b0 + counts[e]] = xsT[:, idx_e[e]]
        wg_c = np.concatenate([tile_gu(gate_proj[e]) for e in (e0, e1)])
        wu_c = np.concatenate([tile_gu(up_proj[e]) for e in (e0, e1)])
        wd_c = np.concatenate([tile_dn(down_proj[e]) for e in (e0, e1)])
        r0, r1 = c * S_SLICE_RAW, (c + 1) * S_SLICE_RAW
        sgp = np.zeros((S_SLICE, H), np.float32)
        sup = np.zeros((S_SLICE, H), np.float32)
        sdp = np.zeros((S_SLICE, H), np.float32)
        sgp[:S_SLICE_RAW] = shared_gate_w[r0:r1, :]
        sup[:S_SLICE_RAW] = shared_up_w[r0:r1, :]
        sdp[:S_SLICE_RAW] = shared_down_w[:, r0:r1].T
        # [S_SLICE, H] -> [MS, P, KH*P] : (m, p_h, ko_h*P + c_si)
        sg_c = np.ascontiguousarray(
            sgp.reshape(MS, P, KH, P).transpose(0, 3, 2, 1)
        ).reshape(MS, P, KH * P).astype(BF16)
        su_c = np.ascontiguousarray(
            sup.reshape(MS, P, KH, P).transpose(0, 3, 2, 1)
        ).reshape(MS, P, KH * P).astype(BF16)
        # [S_SLICE, H] -> [P, MS*H] : (p_si, ko_si*H + c_h)
        sd_c = np.ascontiguousarray(
            sdp.reshape(MS, P, H).transpose(1, 0, 2)
        ).reshape(P, MS * H).astype(BF16)
        in_maps.append({
            "xs": xsT, "xg": xg_c,
            "wg": wg_c, "wu": wu_c, "wd": wd_c,
            "sg": sg_c, "su": su_c, "sd": sd_c,
        })

    nc = _get_compiled(T, caps)
    results = run_bass_kernel_spmd(nc, in_maps, core_ids=list(range(N_CORES)))

    _LAST.clear()
    _LAST.update(nc=nc, in_maps=in_maps, results=results, caps=caps)

    # ---- host-side combine ----
    outT = np.zeros((H, T), np.float32)
    for c in range(N_CORES):
        outT += results.results[c]["ys"].astype(np.float32)
    for c in range(N_CORES):
        ye = results.results[c]["ye"].astype(np.float32)
        for sslot in range(EXP_PER_CORE):
            e = assign[c, sslot]
            cnt = counts[e]
            if cnt == 0:
                continue
            b0 = slot_base[sslot]
            we = comb[idx_e[e], e]
            outT[:, idx_e[e]] += ye[:, b0:b0 + cnt] * we[None, :]

    return np.ascontiguousarray(outT.T).reshape(B, S, H).astype(np.float32)


# revision 25
# speedup vs baseline: 1.0124x; 1.0124x over previous
"""MoE routing kernel for Trainium2 (8 NeuronCores, SPMD expert-parallel).

Contract: kernel(**full_inputs) -> full output [B, S, H] float32.

Strategy
--------
- Host: compute the (tiny) gate + group-topk routing in numpy (bit-identical
  selection to the jax reference), build the per-(token,expert) combine
  weights, and dispatch: gather each expert's tokens into a padded,
  transposed bf16 buffer.  This is the "all-to-all by topk_idx" of the
  sharding hint, done at input-sharding time.
- Device (SPMD over 8 cores): core c holds experts (2c, 2c+1) and a 1/8
  column-slice of the shared expert.  Each core runs the SwiGLU MLP for its
  two experts over their gathered tokens (unweighted), plus its shared
  slice over all tokens, producing partial outputs in [H, tokens] layout.
- Host: scale per-expert outputs by routing weights, scatter-add over
  token indices, add the 8 shared partials, transpose back.

All matmuls run in bf16 with fp32 PSUM accumulation.  Weight panels are
pre-tiled on the host into the exact SBUF tile layout so each streams from
HBM exactly once as a contiguous per-partition DMA; token tiles stay
k-resident; gate/up chains are emitted sequentially per token-slice so
PSUM slots recycle without stalling the PE; DMA issue is spread across the
sync/scalar/gpsimd queues.
"""

import math

import numpy as np
import ml_dtypes

H = 2048          # hidden size
I = 1408          # intermediate per routed expert
E = 16            # routed experts
G = 4             # groups
TOPK_GROUP = 2
TOP_K = 6
N_SHARED = 2
SCALE_FACTOR = 2.5
SI = I * N_SHARED  # 2816 shared intermediate
N_CORES = 8
EXP_PER_CORE = E // N_CORES  # 2
S_SLICE_RAW = SI // N_CORES  # 352
S_SLICE = 384                # padded to 3*128
P = 128
BF16 = ml_dtypes.bfloat16

_COMPILED = {}  # (T, C_cap, w) -> nc
_LAST = {}      # debug/profiling handle for test.py


def _gate_host(hs, gate_weight, bias):
    """numpy replica of reference._gate (verified bit-identical selection)."""
    T = hs.shape[0]
    logits = hs @ gate_weight.T                       # [T, E] fp32
    scores = 1.0 / (1.0 + np.exp(-logits))
    sfc = scores + bias[None, :]
    gs = sfc.reshape(T, G, E // G)
    gsort = np.sort(gs, axis=-1)
    group_scores = gsort[..., -1] + gsort[..., -2]
    group_idx = np.argsort(-group_scores, axis=-1, kind="stable")[:, :TOPK_GROUP]
    gmask = np.zeros((T, G), bool)
    gmask[np.arange(T)[:, None], group_idx] = True
    smask = np.repeat(gmask, E // G, axis=1)
    tmp = np.where(smask, sfc, 0.0)
    topk_idx = np.argsort(-tmp, axis=-1, kind="stable")[:, :TOP_K]
    topk_w = np.take_along_axis(scores, topk_idx, axis=1)
    topk_w = topk_w / (topk_w.sum(-1, keepdims=True) + 1e-20) * SCALE_FACTOR
    return topk_idx.astype(np.int32), topk_w.astype(np.float32)


def _build(T, caps):
    """Build + compile the SPMD Bass program.

    T    : total tokens (every core sees all of them for its shared slice)
    caps : per expert slot, (C_cap, w): gathered-token capacity and matmul
           free-dim slice width; C_cap = NP_R * 2 * w
    """
    import concourse.mybir as mybir
    import concourse.tile as tile
    from concourse import bacc

    bf = mybir.dt.bfloat16
    f32 = mybir.dt.float32
    AF = mybir.ActivationFunctionType

    KH = H // P        # 16 contraction chunks over H
    MI = I // P        # 11 I chunks
    MH = H // P        # 16 output H chunks
    MS = S_SLICE // P  # 3
    NP_S = T // 1024   # shared token blocks (2 x 512 slices each)
    for (C_cap, w) in caps:
        assert C_cap % (2 * w) == 0 and w <= 512
    C_tot = sum(C_cap for C_cap, _ in caps)
    slot_base = [sum(C for C, _ in caps[:s]) for s in range(len(caps))]

    nc = bacc.Bacc("TRN2", target_bir_lowering=False, debug=False,
                   num_devices=N_CORES)
    xs = nc.dram_tensor("xs", [H, T], bf, kind="ExternalInput")
    xg = nc.dram_tensor("xg", [H, C_tot], bf, kind="ExternalInput")
    # weight panels are pre-tiled on the host to the exact SBUF tile layout
    # [tile_idx, partition, ko*128+c] so every load is a contiguous
    # per-partition stream
    wg = nc.dram_tensor("wg", [EXP_PER_CORE * MI, P, KH * P], bf,
                        kind="ExternalInput")
    wu = nc.dram_tensor("wu", [EXP_PER_CORE * MI, P, KH * P], bf,
                        kind="ExternalInput")
    wd = nc.dram_tensor("wd", [EXP_PER_CORE * MH, P, MI * P], bf,
                        kind="ExternalInput")
    sg = nc.dram_tensor("sg", [MS, P, KH * P], bf, kind="ExternalInput")
    su = nc.dram_tensor("su", [MS, P, KH * P], bf, kind="ExternalInput")
    sd = nc.dram_tensor("sd", [P, MS * H], bf, kind="ExternalInput")
    ye = nc.dram_tensor("ye", [H, C_tot], bf, kind="ExternalOutput")
    ys = nc.dram_tensor("ys", [H, T], bf, kind="ExternalOutput")



    MGS = [(0, 4), (4, 4), (8, 3)]          # I chunk groups (11)
    MGS_D = [(0, 4), (4, 4), (8, 4), (12, 4)]  # H chunk groups (16)

    with tile.TileContext(nc) as tc:
        with (
            tc.tile_pool(name="xp", bufs=34) as xp,    # x tiles <=[128,1024] bf16
            tc.tile_pool(name="wp", bufs=6) as wp,     # [128,16,128] weight cols
            tc.tile_pool(name="wdp", bufs=4) as wdp,   # [128,11,128] down cols
            tc.tile_pool(name="sdp", bufs=1) as sdp,   # [128,3,2048] shared down
            tc.tile_pool(name="itp", bufs=46) as itp,  # [128,512] bf16 inter
            tc.tile_pool(name="tmp", bufs=4) as tmp,   # silu temp
            tc.tile_pool(name="otp", bufs=6) as otp,   # [128,1024] bf16 out
            tc.tile_pool(name="pg", bufs=2, space="PSUM") as pgp,
            tc.tile_pool(name="pu", bufs=2, space="PSUM") as pup,
            tc.tile_pool(name="py", bufs=4, space="PSUM") as pyp,
        ):
            # ---------------- shared expert (column slice) ----------------
            sdt = sdp.tile([P, MS, H], bf, name="sdt", tag="sdt")
            nc.scalar.dma_start(sdt[:], sd.ap().rearrange("p (ko c) -> p ko c", c=H))

            # spread the critical first block's loads over four queues so the
            # first matmul chain isn't gated on one sequencer issuing 16 DMAs
            first_engines = [nc.scalar, nc.sync, nc.gpsimd]
            for np_ in range(NP_S):
                c0 = np_ * 1024
                xst = []
                for k in range(KH):
                    t = xp.tile([P, 1024], bf, name=f"xs{np_}_{k}", tag="x")
                    eng = first_engines[k % 3] if np_ == 0 else nc.scalar
                    eng.dma_start(t[:], xs[k * P:(k + 1) * P, c0:c0 + 1024])
                    xst.append(t)
                sint = {}
                for m in range(MS):
                    mo = m * P
                    sgt = wp.tile([P, KH, P], bf, name=f"sgt{np_}_{m}", tag="wp")
                    nc.sync.dma_start(sgt[:], sg[m].rearrange("p (ko c) -> p ko c", c=P))
                    sut = wp.tile([P, KH, P], bf, name=f"sut{np_}_{m}", tag="wp")
                    nc.sync.dma_start(sut[:], su[m].rearrange("p (ko c) -> p ko c", c=P))
                    for j in range(2):
                        psg = pgp.tile([P, 512], f32, name=f"psgs{np_}_{m}{j}",
                                       tag="pg")
                        for k in range(KH):
                            nc.tensor.matmul(psg[:], sgt[:, k, :],
                                             xst[k][:, j * 512:(j + 1) * 512],
                                             start=(k == 0), stop=(k == KH - 1))
                        st = tmp.tile([P, 512], bf, name=f"sts{np_}_{m}{j}",
                                      tag="tmp")
                        nc.scalar.activation(st[:], psg[:], AF.Silu)
                        psu = pup.tile([P, 512], f32, name=f"psus{np_}_{m}{j}",
                                       tag="pu")
                        for k in range(KH):
                            nc.tensor.matmul(psu[:], sut[:, k, :],
                                             xst[k][:, j * 512:(j + 1) * 512],
                                             start=(k == 0), stop=(k == KH - 1))
                        it = itp.tile([P, 512], bf, name=f"si{np_}_{m}{j}",
                                      tag="it")
                        nc.vector.tensor_mul(it[:], st[:], psu[:])
                        sint[(m, j)] = it
                for M in range(MH):
                    ot = otp.tile([P, 1024], bf, name=f"ots{np_}_{M}", tag="ot")
                    for j in range(2):
                        psy = pyp.tile([P, 512], f32, name=f"psys{np_}_{M}{j}",
                                       tag="py")
                        for K in range(MS):
                            nc.tensor.matmul(psy[:], sdt[:, K, M * P:(M + 1) * P],
                                             sint[(K, j)][:],
                                             start=(K == 0), stop=(K == MS - 1))
                        nc.vector.tensor_copy(ot[:, j * 512:(j + 1) * 512], psy[:])
                    nc.gpsimd.dma_start(ys[M * P:(M + 1) * P, c0:c0 + 1024], ot[:])

            # ---------------- routed experts ----------------
            for s, (C_cap, w) in enumerate(caps):
                NP_R = C_cap // (2 * w)
                xgt = {}
                for np_ in range(NP_R):
                    b0 = slot_base[s] + np_ * 2 * w
                    for k in range(KH):
                        t = xp.tile([P, 2 * w], bf, name=f"xg{s}_{np_}_{k}",
                                    tag="x")
                        nc.scalar.dma_start(
                            t[:], xg[k * P:(k + 1) * P, b0:b0 + 2 * w])
                        xgt[(np_, k)] = t
                inter = {}
                for m in range(MI):
                    mo = s * I + m * P
                    wgt = wp.tile([P, KH, P], bf, name=f"wgt{s}_{m}", tag="wp")
                    nc.sync.dma_start(wgt[:], wg[s * MI + m].rearrange("p (ko c) -> p ko c", c=P))
                    wut = wp.tile([P, KH, P], bf, name=f"wut{s}_{m}", tag="wp")
                    nc.sync.dma_start(wut[:], wu[s * MI + m].rearrange("p (ko c) -> p ko c", c=P))
                    for np_ in range(NP_R):
                        for j in range(2):
                            psg = pgp.tile([P, 512], f32,
                                           name=f"psg{s}_{m}_{np_}{j}",
                                           tag="pg")
                            for k in range(KH):
                                nc.tensor.matmul(
                                    psg[:, :w], wgt[:, k, :],
                                    xgt[(np_, k)][:, j * w:(j + 1) * w],
                                    start=(k == 0), stop=(k == KH - 1))
                            st = tmp.tile([P, 512], bf,
                                          name=f"st{s}_{m}_{np_}{j}",
                                          tag="tmp")
                            nc.scalar.activation(st[:, :w], psg[:, :w],
                                                 AF.Silu)
                            psu = pup.tile([P, 512], f32,
                                           name=f"psu{s}_{m}_{np_}{j}",
                                           tag="pu")
                            for k in range(KH):
                                nc.tensor.matmul(
                                    psu[:, :w], wut[:, k, :],
                                    xgt[(np_, k)][:, j * w:(j + 1) * w],
                                    start=(k == 0), stop=(k == KH - 1))
                            it = itp.tile([P, 512], bf,
                                          name=f"it{s}_{m}_{np_}{j}",
                                          tag="it")
                            nc.vector.tensor_mul(it[:, :w], st[:, :w],
                                                 psu[:, :w])
                            inter[(m, np_, j)] = it
                for M in range(MH):
                    Mo = s * H + M * P
                    wdt = wdp.tile([P, MI, P], bf, name=f"wdt{s}_{M}", tag="wdt")
                    nc.sync.dma_start(wdt[:], wd[s * MH + M].rearrange("p (ko c) -> p ko c", c=P))
                    for np_ in range(NP_R):
                        b0 = slot_base[s] + np_ * 2 * w
                        ot = otp.tile([P, 1024], bf,
                                      name=f"ot{s}_{M}_{np_}", tag="ot")
                        for j in range(2):
                            psy = pyp.tile([P, 512], f32,
                                           name=f"psy{s}_{M}_{np_}{j}",
                                           tag="py")
                            for K in range(MI):
                                nc.tensor.matmul(
                                    psy[:, :w], wdt[:, K, :],
                                    inter[(K, np_, j)][:, :w],
                                    start=(K == 0), stop=(K == MI - 1))
                            nc.vector.tensor_copy(
                                ot[:, j * w:(j + 1) * w], psy[:, :w])
                        nc.gpsimd.dma_start(
                            ye[M * P:(M + 1) * P, b0:b0 + 2 * w],
                            ot[:, :2 * w])

    nc.compile()
    return nc


def _get_compiled(T, caps):
    key = (T, tuple(caps))
    if key not in _COMPILED:
        _COMPILED[key] = _build(T, caps)
    return _COMPILED[key]


def _cap_for(maxc):
    maxc = max(int(maxc), 64)
    np_r = max(2, math.ceil(maxc / 2048))
    w = min(512, math.ceil(maxc / (np_r * 2 * 4)) * 4)
    C_cap = np_r * 2 * w
    assert C_cap >= maxc
    return C_cap, w


def kernel(hidden_states, gate_weight, e_score_correction_bias,
           gate_proj, up_proj, down_proj,
           shared_gate_w, shared_up_w, shared_down_w):
    from concourse.bass_utils import run_bass_kernel_spmd

    hs = np.asarray(hidden_states, dtype=np.float32)
    B, S, Hh = hs.shape
    assert Hh == H
    hsf = np.ascontiguousarray(hs.reshape(-1, H))
    T = hsf.shape[0]
    gate_weight = np.asarray(gate_weight, np.float32)
    bias = np.asarray(e_score_correction_bias, np.float32)
    gate_proj = np.asarray(gate_proj, np.float32)
    up_proj = np.asarray(up_proj, np.float32)
    down_proj = np.asarray(down_proj, np.float32)
    shared_gate_w = np.asarray(shared_gate_w, np.float32)
    shared_up_w = np.asarray(shared_up_w, np.float32)
    shared_down_w = np.asarray(shared_down_w, np.float32)

    # ---- routing on host ----
    topk_idx, topk_w = _gate_host(hsf, gate_weight, bias)
    comb = np.zeros((T, E), np.float32)
    np.add.at(comb, (np.arange(T)[:, None], topk_idx), topk_w)
    sel = np.zeros((T, E), bool)
    sel[np.arange(T)[:, None], topk_idx] = True
    idx_e = [np.nonzero(sel[:, e])[0] for e in range(E)]
    counts = np.array([len(ix) for ix in idx_e])

    # assign experts to (core, slot): slot 0 gets the 8 largest, slot 1 the
    # 8 smallest, so each slot's capacity (uniform across cores under SPMD)
    # hugs its own max count
    order = np.argsort(-counts, kind="stable")
    assign = np.zeros((N_CORES, EXP_PER_CORE), np.int64)
    for c in range(N_CORES):
        assign[c, 0] = order[c]
        assign[c, 1] = order[2 * N_CORES - 1 - c]
    caps = [
        _cap_for(counts[assign[:, 0]].max()),
        _cap_for(counts[assign[:, 1]].max()),
    ]
    slot_base = [0, caps[0][0]]
    C_tot = caps[0][0] + caps[1][0]

    # ---- host-side dispatch (shard + transpose + bf16 cast) ----
    xsT = np.ascontiguousarray(hsf.T).astype(BF16)          # [H, T]

    MI, MH, MS, KH = I // P, H // P, S_SLICE // P, H // P

    def tile_gu(wmat):  # [I, H] -> [MI, P, KH*P] : (m, p_h, ko_h*P + c_i)
        return np.ascontiguousarray(
            wmat.reshape(MI, P, KH, P).transpose(0, 3, 2, 1)
        ).reshape(MI, P, KH * P).astype(BF16)

    def tile_dn(wmat):  # [H, I] -> [MH, P, MI*P] : (M, p_i, Ko_i*P + c_h)
        return np.ascontiguousarray(
            wmat.reshape(MH, P, MI, P).transpose(0, 3, 2, 1)
        ).reshape(MH, P, MI * P).astype(BF16)

    in_maps = []
    for c in range(N_CORES):
        e0, e1 = assign[c]
        xg_c = np.zeros((H, C_tot), BF16)
        for sslot, e in enumerate((e0, e1)):
            b0 = slot_base[sslot]
            xg_c[:, b0:b0 + counts[e]] = xsT[:, idx_e[e]]
        wg_c = np.concatenate([tile_gu(gate_proj[e]) for e in (e0, e1)])
        wu_c = np.concatenate([tile_gu(up_proj[e]) for e in (e0, e1)])
        wd_c = np.concatenate([tile_dn(down_proj[e]) for e in (e0, e1)])
        r0, r1 = c * S_SLICE_RAW, (c + 1) * S_SLICE_RAW
        sgp = np.zeros((S_SLICE, H), np.float32)
        sup = np.zeros((S_SLICE, H), np.float32)
        sdp = np.zeros((S_SLICE, H), np.float32)
        sgp[:S_SLICE_RAW] = shared_gate_w[r0:r1, :]
        sup[:S_SLICE_RAW] = shared_up_w[r0:r1, :]
        sdp[:S_SLICE_RAW] = shared_down_w[:, r0:r1].T
        # [S_SLICE, H] -> [MS, P, KH*P] : (m, p_h, ko_h*P + c_si)
        sg_c = np.ascontiguousarray(
            sgp.reshape(MS, P, KH, P).transpose(0, 3, 2, 1)
        ).reshape(MS, P, KH * P).astype(BF16)
        su_c = np.ascontiguousarray(
            sup.reshape(MS, P, KH, P).transpose(0, 3, 2, 1)
        ).reshape(MS, P, KH * P).astype(BF16)
        # [S_SLICE, H] -> [P, MS*H] : (p_si, ko_si*H + c_h)
        sd_c = np.ascontiguousarray(
            sdp.reshape(MS, P, H).transpose(1, 0, 2)
        ).reshape(P, MS * H).astype(BF16)
        in_maps.append({
            "xs": xsT, "xg": xg_c,
            "wg": wg_c, "wu": wu_c, "wd": wd_c,
            "sg": sg_c, "su": su_c, "sd": sd_c,
        })

    nc = _get_compiled(T, caps)
    results = run_bass_kernel_spmd(nc, in_maps, core_ids=list(range(N_CORES)))

    _LAST.clear()
    _LAST.update(nc=nc, in_maps=in_maps, results=results, caps=caps)

    # ---- host-side combine ----
    outT = np.zeros((H, T), np.float32)
    for c in range(N_CORES):
        outT += results.results[c]["ys"].astype(np.float32)
    for c in range(N_CORES):
        ye = results.results[c]["ye"].astype(np.float32)
        for sslot in range(EXP_PER_CORE):
            e = assign[c, sslot]
            cnt = counts[e]
            if cnt == 0:
                continue
            b0 = slot_base[sslot]
            we = comb[idx_e[e], e]
            outT[:, idx_e[e]] += ye[:, b0:b0 + cnt] * we[None, :]

    return np.ascontiguousarray(outT.T).reshape(B, S, H).astype(np.float32)
